# revision 29
# baseline (speedup 1.0000x reference)
"""Single-head causal attention (B=4, S=2048, D=1024, dk=128) on 8 TRN2 cores.

Sharding: core c -> batch b=c//2, half h=c%2.
  - h=0 handles query rows [0:512) u [1536:2048), h=1 handles [512:1536)
    (balances causal work: 4+16 vs 8+12 key-tiles per 512-query block).
  - Each core projects the full K/V for its batch (cheaper than an
    intra-pair collective exchange, which measures ~36us on HW).

Precision: qx/kx/wq/wk are fp8e4m3 (halves the score-path HBM bytes;
fp8 x fp8 matmuls run at bf16 rate - a mixed bf16 x fp8 matmul lowers
to half rate), vx and wv stay bf16: fp8 on the V path alone costs
~2.3e-2 max-rel error, over the 2e-2 budget, while the fp8 score path
measures 1.79e-2.  wq/wk are pre-scaled x16 into fp8's normal range;
the x256 score inflation is folded into the exp scale.

Layout: the host pre-marshals every tensor into the exact [partition,
chunk, col] block layout the SBUF tiles use, so each block loads as
one DMA with 2-8 KB contiguous runs per partition (per-queue DMA
throughput is descriptor-rate-limited: 512 B runs cap a queue near
50 GB/s).  Every DMA block gets its OWN SBUF tile and every projection
block its own output tile: dependency tracking is tile-granular, so a
shared tile would make early consumers wait for the last writer.

DMA: three queues (sync/scalar HWDGE + gpsimd SWDGE) see 70-170 GB/s
each depending on chip-wide contention from the other 7 cores, so the
critical-path tensors (wqk, qx, kx pieces) sit at the heads of all
three queues and the 4 MB of V is spread across all three tails.
HWDGE rings hold ~4 outstanding descriptors - a 5th dma_start blocks
the issuing engine - so scalar (which must run the exp chain) issues
only 4 loads up front and one more mid-chain.

Compute: projections contract d_model on the partition dim and emit
qT/kT [dk, s] directly.  Scores are computed transposed ([key, query])
so the P@V matmul consumes P tiles as the stationary operand (FWL
makes the per-tile LDWEIGHTS ~free) and V in natural [s, dk] layout; a
ones-column appended to V makes the same matmul accumulate the softmax
denominators.  Score PSUM tiles span two banks [128, 1024] (two key
tiles) so one ACTIVATE exps both - the serial ACT chain is the
critical path and each ACTIVATE carries a 352-cycle fixed overhead.
The causal mask is applied as a multiplicative bf16 mask on P,
generated on-chip from a per-core [128, 16] shift table (pairs of key
tiles per compare via an offset iota) so all 8 cores run one identical
program.  Output is stored per 512-row block in (p q) k layout = 2 KB
contiguous per partition row.
"""

import math

import numpy as np
import ml_dtypes

import concourse.bacc as bacc
import concourse.tile as tile
import concourse.mybir as mybir
from concourse import bass_utils
from concourse.masks import make_identity
from concourse.tile_rust import add_dep_helper

F32 = mybir.dt.float32
BF16 = mybir.dt.bfloat16
FP8 = mybir.dt.float8e4

B, S, DM, DK = 4, 2048, 1024, 128
NCORES = 8
HALF = S // 2  # query rows per core / key columns per pipeline stage
NCH = DM // 128  # d_model chunks
# program-wide causal shape: query block 0 sees key tiles [0, NJ0),
# block 1 sees [0, NJ1); per-core mask data zeroes what's invalid.
NJ0, NJ1 = 8, 16
VW = DK + 1  # v tiles carry a ones-column for the softmax denominator
WSC = 16.0  # wq/wk pre-scaled into fp8's normal range
SCALE = 1.0 / (math.sqrt(DK) * WSC * WSC)
WARMUP_MMS = 26
FILLER_MMS = 20

_CACHE = {}


def _build():
    if "nc" in _CACHE:
        return _CACHE["nc"]
    nc = bacc.Bacc("TRN2", target_bir_lowering=False, debug=False, num_devices=NCORES)

    # activations pre-blocked host-side to match SBUF tiles exactly
    qx_in = nc.dram_tensor("qx", [128, 2, NCH, 512], FP8, kind="ExternalInput").ap()
    kx_in = nc.dram_tensor("kx", [128, 8, NCH, 256], FP8, kind="ExternalInput").ap()
    vx_in = nc.dram_tensor("vx", [128, 4, NCH, 512], BF16, kind="ExternalInput").ap()
    wqk_in = nc.dram_tensor("wqk", [128, NCH, 2 * DK], FP8, kind="ExternalInput").ap()
    wv_in = nc.dram_tensor("wv", [128, NCH, DK], BF16, kind="ExternalInput").ap()
    shifts_in = nc.dram_tensor("shifts", [128, 16], F32, kind="ExternalInput").ap()
    out = nc.dram_tensor("out", [HALF, DK], F32, kind="ExternalOutput").ap()

    with tile.TileContext(nc) as tc:
        with tc.tile_pool(name="const", bufs=1) as const:
            wqk = const.tile([128, NCH, 2 * DK], FP8, tag="wqk", name="wqk")
            wv = const.tile([128, NCH, DK], BF16, tag="wv", name="wv")
            shifts = const.tile([128, 16], F32)
            qxb = [const.tile([128, NCH, 512], FP8, tag=f"qx{b}", name=f"qx{b}")
                   for b in range(2)]
            kxp = [const.tile([128, NCH, 256], FP8, tag=f"kx{b}", name=f"kx{b}")
                   for b in range(8)]
            vxb = [const.tile([128, NCH, 512], BF16, tag=f"vx{b}", name=f"vx{b}")
                   for b in range(4)]

            w_warm = const.tile([128, 512], BF16)
            nc.vector.memset(w_warm, 1.0)

            # ---- loads (see module docstring for the queue strategy)
            nc.scalar.dma_start(out=wqk, in_=wqk_in)
            nc.scalar.dma_start(out=qxb[0], in_=qx_in[:, 0])
            nc.scalar.dma_start(out=kxp[2], in_=kx_in[:, 2])
            nc.scalar.dma_start(out=kxp[4], in_=kx_in[:, 4])
            nc.scalar.dma_start(out=kxp[7], in_=kx_in[:, 7])
            # + vx1 issued mid-pipeline below

            nc.sync.dma_start(out=shifts, in_=shifts_in)
            nc.sync.dma_start(out=kxp[0], in_=kx_in[:, 0])
            nc.sync.dma_start(out=qxb[1], in_=qx_in[:, 1])
            nc.sync.dma_start(out=kxp[3], in_=kx_in[:, 3])
            nc.sync.dma_start(out=vxb[0], in_=vx_in[:, 0])
            nc.sync.dma_start(out=vxb[1][:, 0:4, :], in_=vx_in[:, 1, 0:4])
            nc.sync.dma_start(out=vxb[3][:, 0:4, :], in_=vx_in[:, 3, 0:4])

            nc.gpsimd.dma_start(out=kxp[1], in_=kx_in[:, 1])
            nc.gpsimd.dma_start(out=kxp[6], in_=kx_in[:, 6])
            nc.gpsimd.dma_start(out=kxp[5], in_=kx_in[:, 5])
            nc.gpsimd.dma_start(out=wv, in_=wv_in)
            nc.gpsimd.dma_start(out=vxb[1][:, 4:8, :], in_=vx_in[:, 1, 4:8])
            nc.gpsimd.dma_start(out=vxb[2], in_=vx_in[:, 2])
            nc.gpsimd.dma_start(out=vxb[3][:, 4:8, :], in_=vx_in[:, 3, 4:8])

            # gpsimd auxiliary ops AFTER its dma issues (SWDGE descriptor
            # generation runs on the engine and must not be delayed)
            iota_i = const.tile([128, 1024], mybir.dt.int32)
            nc.gpsimd.iota(iota_i[:, 0:512], pattern=[[1, 512]], base=0,
                           channel_multiplier=0)
            nc.gpsimd.iota(iota_i[:, 512:1024], pattern=[[1, 512]], base=-128,
                           channel_multiplier=0)

            ident = const.tile([128, 128], BF16)
            make_identity(nc, ident)

            # ---- causal masks: mask[p, t, c] = (c >= shift[p, t]).
            # shift[t+1] = shift[t] + 128, so one compare against an offset
            # iota produces the (t, t+1) pair in a single [128, 1024] op.
            # fp16 tensor operand (integers <= 2047 are exact) for 2x DVE
            # rate; generated in two batches between the kT casts they don't
            # block.
            iota2 = const.tile([128, 1024], mybir.dt.float16)
            nc.vector.tensor_copy(iota2, iota_i)
            masks_sb = {}

            def gen_masks(ts):
                for t in ts:
                    masks_sb[t] = const.tile([128, 1024], BF16, tag=f"mask{t}",
                                             name=f"mask{t}")
                    nc.vector.tensor_scalar(
                        masks_sb[t],
                        iota2,
                        shifts[:, t : t + 1],
                        None,
                        op0=mybir.AluOpType.is_ge,
                    )

            # ---- PE warmup + filler: dummy matmuls keep the HAM clock-gate
            # open while the PE waits for the first loads.  (Uses w_warm, not
            # the identity, so warmup starts before gpsimd finishes its DMA
            # issue work.)
            last_filler = None
            with tc.tile_pool(name="psW", bufs=1, space="PSUM") as psW:
                ps_w = psW.tile([128, 512], F32)
                for _ in range(WARMUP_MMS):
                    nc.tensor.matmul(
                        ps_w[:, 0:128], w_warm[:, 0:128], w_warm[:, 0:128],
                        start=True, stop=True
                    )
                for _ in range(FILLER_MMS):
                    last_filler = nc.tensor.matmul(
                        ps_w, w_warm[:, 0:128], w_warm, start=True, stop=True
                    )

            # ---- projected tensors: one tile per writer so tile-granular
            # dependency tracking never over-serializes ----
            qTb = [const.tile([128, 512], BF16, tag=f"qT{b}", name=f"qT{b}")
                   for b in range(2)]
            kT = [const.tile([128, 256], BF16, tag=f"kT{b}", name=f"kT{b}")
                  for b in range(8)]
            vTq = [const.tile([128, 512], BF16, tag=f"vT{b}", name=f"vT{b}")
                   for b in range(4)]
            vsbh = [const.tile([128, NCH, VW], BF16, tag=f"v{h}", name=f"vsb{h}") for h in range(2)]

            with (
                tc.tile_pool(name="psM", bufs=2, space="PSUM") as psM,
                tc.tile_pool(name="psS", bufs=2, space="PSUM") as psS,
                tc.tile_pool(name="psO", bufs=2, space="PSUM") as psO,
                tc.tile_pool(name="pP", bufs=14) as p_pool,
                tc.tile_pool(name="oo", bufs=4) as o_pool,
            ):

                def project_block(wT, k0, xblk, dst, w=512):
                    """dst bf16 = W[k0 cols] @ xblk (contract d_model chunks)."""
                    acc = psM.tile([128, 512], F32, tag="ps_misc", name="acc")
                    for c in range(NCH):
                        mm = nc.tensor.matmul(
                            acc[:, 0:w],
                            wT[:, c, k0 : k0 + DK],
                            xblk[:, c, :],
                            start=(c == 0),
                            stop=(c == NCH - 1),
                        )
                        if c == 0 and last_filler is not None:
                            add_dep_helper(
                                mm.ins, last_filler.ins, sync=False,
                                reason="run filler first",
                            )
                    nc.vector.tensor_copy(dst, acc[:, 0:w])

                def project_dr(wT, k0, xblk, dst, w=512):
                    """fp8 DoubleRow projection: chunk PAIRS per matmul."""
                    acc = psM.tile([128, 512], F32, tag="ps_misc", name="acc")
                    for u in range(NCH // 2):
                        mm = nc.tensor.matmul(
                            acc[:, 0:w],
                            wT[:, 2 * u : 2 * u + 2, k0 : k0 + DK],
                            xblk[:, 2 * u : 2 * u + 2, :],
                            start=(u == 0),
                            stop=(u == NCH // 2 - 1),
                            perf_mode=mybir.MatmulPerfMode.DoubleRow,
                        )
                        if u == 0 and last_filler is not None:
                            add_dep_helper(
                                mm.ins, last_filler.ins, sync=False,
                                reason="run filler first",
                            )
                    nc.vector.tensor_copy(dst, acc[:, 0:w])

                def scores_pair(blk, j, masked):
                    """exp(score) for key tiles (j, j+1) x 512 queries of blk.

                    One [128, 1024] PSUM pair, one ACTIVATE, optional mask
                    multiply.  Returns the bf16 p pair tile."""
                    ps_s = psS.tile([128, 1024], F32, tag="score")
                    for i in range(2):
                        jl = j + i
                        nc.tensor.matmul(
                            ps_s[:, i * 512 : (i + 1) * 512],
                            kT[jl // 2][:, (jl % 2) * 128 : (jl % 2 + 1) * 128],
                            qTb[blk],
                            start=True,
                            stop=True,
                        )
                    p_t = p_pool.tile([128, 1024], BF16, tag="p")
                    nc.scalar.activation(
                        p_t, ps_s, mybir.ActivationFunctionType.Exp, scale=SCALE
                    )
                    return p_t

                def v_quarter(q):
                    """project + transpose vx quarter q into vsbh[q//2]."""
                    h = q // 2
                    if q % 2 == 0:
                        nc.vector.memset(vsbh[h][:, :, DK : DK + 1], 1.0)
                    project_block(wv, 0, vxb[q], vTq[q])
                    t0 = (q % 2) * 4
                    ps = psM.tile([128, 4, 128], BF16, tag="ps_misc")
                    for tl in range(4):
                        nc.tensor.transpose(
                            ps[:, tl, :], vTq[q][:, tl * 128 : (tl + 1) * 128],
                            ident
                        )
                    nc.vector.tensor_copy(vsbh[h][:, t0 : t0 + 4, 0:DK], ps)

                o_big = [
                    o_pool.tile([128, 4, DK], F32, tag=f"ob{b}", name=f"ob{b}", bufs=1)
                    for b in range(2)
                ]
                out4 = out.rearrange("(b p q) k -> b p q k", q=4, p=128)

                def div_out(blk, qs, ps_o):
                    rec = o_pool.tile([128, 1], F32, tag="rec")
                    nc.vector.reciprocal(rec, ps_o[:, DK : DK + 1])
                    nc.vector.tensor_scalar_mul(o_big[blk][:, qs, :], ps_o[:, 0:DK], rec)
                    if qs == 3 and blk == 0:
                        nc.sync.dma_start(out=out4[0], in_=o_big[0])

                def pv(ps_o, p_pairs, qs, jset, h, start, stop):
                    j0 = jset[0] if isinstance(jset, list) else jset.start
                    for n, j in enumerate(jset):
                        nc.tensor.matmul(
                            ps_o,
                            p_pairs[(j - j0) // 2][
                                :, (j % 2) * 512 + qs * 128 : (j % 2) * 512 + (qs + 1) * 128
                            ],
                            vsbh[h][:, j % NCH, :],
                            start=(start and n == 0),
                            stop=(stop and n == len(jset) - 1),
                        )

                # ---------- pipeline ----------
                # tile_wait_until hints tell the static scheduler when each
                # stage's DMA data really lands (measured ETAs), so the
                # in-order engine streams don't park early-emitted work in
                # front of later work whose inputs arrive sooner.
                W = tc.tile_wait_until
                with nc.named_scope("q_proj"):
                    with W(0.007):
                        project_dr(wqk, 0, qxb[0], qTb[0])
                with W(0.0065):
                    project_dr(wqk, DK, kxp[0], kT[0], w=256)
                p0, p1, p1b = [], [], []
                with nc.named_scope("sc_a"):
                    with W(0.0075):
                        p0.append(scores_pair(0, 0, True))
                        gen_masks([0, 2])
                    with W(0.008):
                        project_dr(wqk, 0, qxb[1], qTb[1])
                        project_dr(wqk, DK, kxp[1], kT[1], w=256)
                    with W(0.0085):
                        p1.append(scores_pair(1, 0, False))
                    with W(0.009):
                        project_dr(wqk, DK, kxp[2], kT[2], w=256)
                        p0.append(scores_pair(0, 2, True))
                        gen_masks([4, 6])
                    with W(0.010):
                        p1.append(scores_pair(1, 2, False))
                with nc.named_scope("sc_b"):
                    with W(0.0105):
                        project_dr(wqk, DK, kxp[3], kT[3], w=256)
                        p0.append(scores_pair(0, 4, True))
                    with W(0.0115):
                        p1.append(scores_pair(1, 4, False))
                    with W(0.012):
                        project_dr(wqk, DK, kxp[4], kT[4], w=256)
                        p0.append(scores_pair(0, 6, True))
                    with W(0.013):
                        p1.append(scores_pair(1, 6, False))
                    # mask multiplies offloaded to gpsimd: the DVE is the
                    # co-bottleneck in this window (kT casts + vq copies +
                    # divs); gpsimd is idle once SWDGE descgen drains
                    with W(0.018):
                        for n in range(4):
                            nc.gpsimd.tensor_mul(p0[n], p0[n], masks_sb[2 * n])
                ps_o0 = [psO.tile([128, VW], F32, tag="out", name=f"ps_o0_{i}") for i in range(4)]
                ps_o1 = [psO.tile([128, VW], F32, tag="out", name=f"ps_o1_{i}") for i in range(4)]
                with nc.named_scope("sc_c"):
                    with W(0.013):
                        project_dr(wqk, DK, kxp[5], kT[5], w=256)
                    with W(0.014):
                        p1b.append(scores_pair(1, 8, True))
                        gen_masks([8, 10, 12, 14])
                    with W(0.0145):
                        project_dr(wqk, DK, kxp[6], kT[6], w=256)
                    with W(0.015):
                        p1b.append(scores_pair(1, 10, True))
                    with W(0.016):
                        project_dr(wqk, DK, kxp[7], kT[7], w=256)
                    with W(0.017):
                        p1b.append(scores_pair(1, 12, True))
                    with W(0.018):
                        p1b.append(scores_pair(1, 14, True))
                    with W(0.022):
                        for n in range(2):
                            nc.gpsimd.tensor_mul(p1b[n], p1b[n], masks_sb[8 + 2 * n])
                        for n in range(2, 4):
                            nc.vector.tensor_mul(p1b[n], p1b[n], masks_sb[8 + 2 * n])
                with nc.named_scope("vnat"):
                    with W(0.011):
                        v_quarter(0)
                    with W(0.012):
                        v_quarter(1)
                with nc.named_scope("pv01"):
                    with W(0.013):
                        for qs in range(4):
                            pv(ps_o0[qs], p0, qs, range(NJ0), 0, True, True)
                            div_out(0, qs, ps_o0[qs])
                with nc.named_scope("vnat2"):
                    with W(0.0135):
                        v_quarter(2)
                with nc.named_scope("pv1"):
                    with W(0.0145):
                        for qs in range(4):
                            pv(ps_o1[qs], p1, qs, range(NCH), 0, True, False)
                with nc.named_scope("vnat3"):
                    with W(0.015):
                        v_quarter(3)
                with nc.named_scope("pv1b"):
                    with W(0.018):
                        for qs in range(4):
                            pv(ps_o1[qs], p1b, qs, range(NCH, NJ1), 1, False, True)
                            div_out(1, qs, ps_o1[qs])
                with nc.named_scope("store1"):
                    nc.scalar.dma_start(out=out4[1, :, 0:2], in_=o_big[1][:, 0:2])
                    nc.sync.dma_start(out=out4[1, :, 2:4], in_=o_big[1][:, 2:4])

    nc.compile()
    _CACHE["nc"] = nc
    return nc


def _shift_block(h):
    """[128, 16] f32: mask[p, t, c] = (c >= shift) == (key 128t+p <= query qb+c)."""
    qbase = (0, 1536) if h == 0 else (512, 1024)
    p = np.arange(128, dtype=np.float32)[:, None]
    t = np.arange(16, dtype=np.float32)[None, :]
    qb = np.where(t < NJ0, qbase[0], qbase[1])
    return (128.0 * t + p - qb).astype(np.float32)


def _blocked(arr, nblk, dtype):
    """[DM, ncols] -> [128, nblk, NCH, ncols//nblk] matching the SBUF tiles."""
    w = arr.shape[1] // nblk
    return np.ascontiguousarray(
        arr.reshape(NCH, 128, nblk, w).transpose(1, 2, 0, 3)
    ).astype(dtype)


def kernel(**inputs):
    queries = np.asarray(inputs["queries"], dtype=np.float32)
    keys = np.asarray(inputs["keys"], dtype=np.float32)
    values = np.asarray(inputs["values"], dtype=np.float32)

    nc = _build()
    f8 = ml_dtypes.float8_e4m3fn
    bf = ml_dtypes.bfloat16
    shifts = [_shift_block(0), _shift_block(1)]
    qrows = [np.r_[0:512, 1536:2048], np.r_[512:1536]]
    wT = {
        nm: np.asarray(inputs[nm], dtype=np.float32).T
        for nm in ("Wq", "Wk", "Wv")
    }
    wqk = np.ascontiguousarray(
        np.concatenate([wT["Wq"], wT["Wk"]], axis=1).reshape(NCH, 128, 2 * DK)
        .transpose(1, 0, 2) * WSC
    ).astype(f8)
    wv = np.ascontiguousarray(
        wT["Wv"].reshape(NCH, 128, DK).transpose(1, 0, 2)
    ).astype(bf)
    kxs = [_blocked(keys[b].T, 8, f8) for b in range(B)]
    vxs = [_blocked(values[b].T, 4, bf) for b in range(B)]

    in_maps = []
    for c in range(NCORES):
        b, h = c // 2, c % 2
        in_maps.append(
            {
                "qx": _blocked(queries[b][qrows[h]].T, 2, f8),
                "kx": kxs[b],
                "vx": vxs[b],
                "wqk": wqk,
                "wv": wv,
                "shifts": shifts[h],
            }
        )

    res = bass_utils.run_bass_kernel_spmd(
        nc, in_maps, list(range(NCORES)), **_CACHE.get("run_kwargs", {})
    )
    _CACHE["last_result"] = res

    # store layout is (p q): dram row blk*512 + p*4 + qs <- query qs*128 + p
    r = np.arange(512)
    local_q = (r % 4) * 128 + r // 4  # query index within block at dram row r
    perm = np.concatenate([local_q, 512 + local_q])
    out = np.empty((B, S, DK), dtype=np.float32)
    for c in range(NCORES):
        b, h = c // 2, c % 2
        out[b][qrows[h][perm]] = res.results[c]["out"]
    return out


# revision 31
# speedup vs baseline: 1.1915x; 1.1915x over previous
"""Single-head causal attention (B=4, S=2048, D=1024, dk=128) on 8 TRN2 cores.

Sharding: core c -> batch b=c//2, half h=c%2.
  - h=0 handles query rows [0:512) u [1536:2048), h=1 handles [512:1536)
    (balances causal work: 4+16 vs 8+12 key-tiles per 512-query block).
  - Each core projects the full K/V for its batch (cheaper than an
    intra-pair collective exchange, which measures ~36us on HW).

Precision: qx/kx/wq/wk are fp8e4m3 (halves the score-path HBM bytes;
fp8 x fp8 matmuls run at bf16 rate - a mixed bf16 x fp8 matmul lowers
to half rate), vx and wv stay bf16: fp8 on the V path alone costs
~2.3e-2 max-rel error, over the 2e-2 budget, while the fp8 score path
measures 1.79e-2.  wq/wk are pre-scaled x16 into fp8's normal range;
the x256 score inflation is folded into the exp scale.

Layout: the host pre-marshals every tensor into the exact [partition,
chunk, col] block layout the SBUF tiles use, so each block loads as
one DMA with 2-8 KB contiguous runs per partition (per-queue DMA
throughput is descriptor-rate-limited: 512 B runs cap a queue near
50 GB/s).  Every DMA block gets its OWN SBUF tile and every projection
block its own output tile: dependency tracking is tile-granular, so a
shared tile would make early consumers wait for the last writer.

DMA: three queues (sync/scalar HWDGE + gpsimd SWDGE) see 70-170 GB/s
each depending on chip-wide contention from the other 7 cores, so the
critical-path tensors (wqk, qx, kx pieces) sit at the heads of all
three queues and the 4 MB of V is spread across all three tails.
HWDGE rings hold ~4 outstanding descriptors - a 5th dma_start blocks
the issuing engine - so scalar (which must run the exp chain) issues
only 4 loads up front and one more mid-chain.

Compute: projections contract d_model on the partition dim and emit
qT/kT [dk, s] directly.  Scores are computed transposed ([key, query])
so the P@V matmul consumes P tiles as the stationary operand (FWL
makes the per-tile LDWEIGHTS ~free) and V in natural [s, dk] layout; a
ones-column appended to V makes the same matmul accumulate the softmax
denominators.  Score PSUM tiles span two banks [128, 1024] (two key
tiles) so one ACTIVATE exps both - the serial ACT chain is the
critical path and each ACTIVATE carries a 352-cycle fixed overhead.
The causal mask is applied as a multiplicative bf16 mask on P,
generated on-chip from a per-core [128, 16] shift table (pairs of key
tiles per compare via an offset iota) so all 8 cores run one identical
program.  Output is stored per 512-row block in (p q) k layout = 2 KB
contiguous per partition row.
"""

import math

import numpy as np
import ml_dtypes

import concourse.bacc as bacc
import concourse.tile as tile
import concourse.mybir as mybir
from concourse import bass_utils
from concourse.masks import make_identity
from concourse.tile_rust import add_dep_helper

F32 = mybir.dt.float32
BF16 = mybir.dt.bfloat16
FP8 = mybir.dt.float8e4

B, S, DM, DK = 4, 2048, 1024, 128
NCORES = 8
HALF = S // 2  # query rows per core / key columns per pipeline stage
NCH = DM // 128  # d_model chunks
# program-wide causal shape: query block 0 sees key tiles [0, NJ0),
# block 1 sees [0, NJ1); per-core mask data zeroes what's invalid.
NJ0, NJ1 = 8, 16
VW = DK + 1  # v tiles carry a ones-column for the softmax denominator
WSC = 16.0  # wq/wk pre-scaled into fp8's normal range
SCALE = 1.0 / (math.sqrt(DK) * WSC * WSC)
WARMUP_MMS = 26
FILLER_MMS = 20

_CACHE = {}


def _build():
    if "nc" in _CACHE:
        return _CACHE["nc"]
    nc = bacc.Bacc("TRN2", target_bir_lowering=False, debug=False, num_devices=NCORES)

    # activations pre-blocked host-side to match SBUF tiles exactly
    qx_in = nc.dram_tensor("qx", [128, 2, NCH, 512], FP8, kind="ExternalInput").ap()
    kx_in = nc.dram_tensor("kx", [128, 8, NCH, 256], FP8, kind="ExternalInput").ap()
    vx_in = nc.dram_tensor("vx", [128, 4, NCH, 512], BF16, kind="ExternalInput").ap()
    wqk_in = nc.dram_tensor("wqk", [128, NCH, 2 * DK], FP8, kind="ExternalInput").ap()
    wv_in = nc.dram_tensor("wv", [128, NCH, DK], BF16, kind="ExternalInput").ap()
    shifts_in = nc.dram_tensor("shifts", [128, 16], F32, kind="ExternalInput").ap()
    out = nc.dram_tensor("out", [HALF, DK], F32, kind="ExternalOutput").ap()

    with tile.TileContext(nc) as tc:
        with tc.tile_pool(name="const", bufs=1) as const:
            wqk = const.tile([128, NCH, 2 * DK], FP8, tag="wqk", name="wqk")
            wv = const.tile([128, NCH, DK], BF16, tag="wv", name="wv")
            shifts = const.tile([128, 16], F32)
            qxb = [const.tile([128, NCH, 512], FP8, tag=f"qx{b}", name=f"qx{b}")
                   for b in range(2)]
            kxp = [const.tile([128, NCH, 256], FP8, tag=f"kx{b}", name=f"kx{b}")
                   for b in range(8)]
            vxb = [const.tile([128, NCH, 512], BF16, tag=f"vx{b}", name=f"vx{b}")
                   for b in range(4)]

            w_warm = const.tile([128, 512], BF16)
            nc.vector.memset(w_warm, 1.0)

            # ---- loads (see module docstring for the queue strategy)
            nc.scalar.dma_start(out=wqk, in_=wqk_in)
            nc.scalar.dma_start(out=qxb[0], in_=qx_in[:, 0])
            nc.scalar.dma_start(out=kxp[2], in_=kx_in[:, 2])
            nc.scalar.dma_start(out=kxp[4], in_=kx_in[:, 4])
            nc.scalar.dma_start(out=kxp[7], in_=kx_in[:, 7])
            # + vx1 issued mid-pipeline below

            nc.sync.dma_start(out=shifts, in_=shifts_in)
            nc.sync.dma_start(out=kxp[0], in_=kx_in[:, 0])
            nc.sync.dma_start(out=qxb[1], in_=qx_in[:, 1])
            nc.sync.dma_start(out=vxb[0], in_=vx_in[:, 0])
            nc.sync.dma_start(out=vxb[1][:, 4:8, :], in_=vx_in[:, 1, 4:8])
            nc.sync.dma_start(out=vxb[3][:, 0:4, :], in_=vx_in[:, 3, 0:4])

            nc.gpsimd.dma_start(out=kxp[1], in_=kx_in[:, 1])
            nc.gpsimd.dma_start(out=kxp[6], in_=kx_in[:, 6])
            nc.gpsimd.dma_start(out=kxp[5], in_=kx_in[:, 5])
            nc.gpsimd.dma_start(out=kxp[3], in_=kx_in[:, 3])
            nc.gpsimd.dma_start(out=wv, in_=wv_in)
            nc.gpsimd.dma_start(out=vxb[2], in_=vx_in[:, 2])
            nc.gpsimd.dma_start(out=vxb[3][:, 4:8, :], in_=vx_in[:, 3, 4:8])

            # gpsimd auxiliary ops AFTER its dma issues (SWDGE descriptor
            # generation runs on the engine and must not be delayed)
            iota_i = const.tile([128, 1024], mybir.dt.int32)
            nc.gpsimd.iota(iota_i[:, 0:512], pattern=[[1, 512]], base=0,
                           channel_multiplier=0)
            nc.gpsimd.iota(iota_i[:, 512:1024], pattern=[[1, 512]], base=-128,
                           channel_multiplier=0)

            ident = const.tile([128, 128], BF16)
            make_identity(nc, ident)

            # ---- causal masks: mask[p, t, c] = (c >= shift[p, t]).
            # shift[t+1] = shift[t] + 128, so one compare against an offset
            # iota produces the (t, t+1) pair in a single [128, 1024] op.
            # fp16 tensor operand (integers <= 2047 are exact) for 2x DVE
            # rate; generated in two batches between the kT casts they don't
            # block.
            iota2 = const.tile([128, 1024], mybir.dt.float16)
            nc.vector.tensor_copy(iota2, iota_i)
            masks_sb = {}

            def gen_masks(ts):
                for t in ts:
                    masks_sb[t] = const.tile([128, 1024], BF16, tag=f"mask{t}",
                                             name=f"mask{t}")
                    nc.vector.tensor_scalar(
                        masks_sb[t],
                        iota2,
                        shifts[:, t : t + 1],
                        None,
                        op0=mybir.AluOpType.is_ge,
                    )

            # ---- PE warmup + filler: dummy matmuls keep the HAM clock-gate
            # open while the PE waits for the first loads.  (Uses w_warm, not
            # the identity, so warmup starts before gpsimd finishes its DMA
            # issue work.)
            last_filler = None
            with tc.tile_pool(name="psW", bufs=1, space="PSUM") as psW:
                ps_w = psW.tile([128, 512], F32)
                for _ in range(WARMUP_MMS):
                    nc.tensor.matmul(
                        ps_w[:, 0:128], w_warm[:, 0:128], w_warm[:, 0:128],
                        start=True, stop=True
                    )
                for _ in range(FILLER_MMS):
                    last_filler = nc.tensor.matmul(
                        ps_w, w_warm[:, 0:128], w_warm, start=True, stop=True
                    )

            # ---- projected tensors: one tile per writer so tile-granular
            # dependency tracking never over-serializes ----
            qTb = [const.tile([128, 512], BF16, tag=f"qT{b}", name=f"qT{b}")
                   for b in range(2)]
            kT = [const.tile([128, 256], BF16, tag=f"kT{b}", name=f"kT{b}")
                  for b in range(8)]
            vTq = [const.tile([128, 512], BF16, tag=f"vT{b}", name=f"vT{b}")
                   for b in range(4)]
            vsbh = [const.tile([128, NCH, VW], BF16, tag=f"v{h}", name=f"vsb{h}") for h in range(2)]

            with (
                tc.tile_pool(name="psM", bufs=2, space="PSUM") as psM,
                tc.tile_pool(name="psS", bufs=2, space="PSUM") as psS,
                tc.tile_pool(name="psO", bufs=2, space="PSUM") as psO,
                tc.tile_pool(name="pP", bufs=14) as p_pool,
                tc.tile_pool(name="oo", bufs=4) as o_pool,
            ):

                def project_block(wT, k0, xblk, dst, w=512):
                    """dst bf16 = W[k0 cols] @ xblk (contract d_model chunks)."""
                    acc = psM.tile([128, 512], F32, tag="ps_misc", name="acc")
                    for c in range(NCH):
                        mm = nc.tensor.matmul(
                            acc[:, 0:w],
                            wT[:, c, k0 : k0 + DK],
                            xblk[:, c, :],
                            start=(c == 0),
                            stop=(c == NCH - 1),
                        )
                        if c == 0 and last_filler is not None:
                            add_dep_helper(
                                mm.ins, last_filler.ins, sync=False,
                                reason="run filler first",
                            )
                    nc.vector.tensor_copy(dst, acc[:, 0:w])

                def project_dr(wT, k0, xblk, dst, w=512):
                    """fp8 DoubleRow projection: chunk PAIRS per matmul."""
                    acc = psM.tile([128, 512], F32, tag="ps_misc", name="acc")
                    for u in range(NCH // 2):
                        mm = nc.tensor.matmul(
                            acc[:, 0:w],
                            wT[:, 2 * u : 2 * u + 2, k0 : k0 + DK],
                            xblk[:, 2 * u : 2 * u + 2, :],
                            start=(u == 0),
                            stop=(u == NCH // 2 - 1),
                            perf_mode=mybir.MatmulPerfMode.DoubleRow,
                        )
                        if u == 0 and last_filler is not None:
                            add_dep_helper(
                                mm.ins, last_filler.ins, sync=False,
                                reason="run filler first",
                            )
                    nc.vector.tensor_copy(dst, acc[:, 0:w])

                def scores_pair(blk, j, masked):
                    """exp(score) for key tiles (j, j+1) x 512 queries of blk.

                    One [128, 1024] PSUM pair, one ACTIVATE, optional mask
                    multiply.  Returns the bf16 p pair tile."""
                    ps_s = psS.tile([128, 1024], F32, tag="score")
                    for i in range(2):
                        jl = j + i
                        nc.tensor.matmul(
                            ps_s[:, i * 512 : (i + 1) * 512],
                            kT[jl // 2][:, (jl % 2) * 128 : (jl % 2 + 1) * 128],
                            qTb[blk],
                            start=True,
                            stop=True,
                        )
                    p_t = p_pool.tile([128, 1024], BF16, tag="p")
                    nc.scalar.activation(
                        p_t, ps_s, mybir.ActivationFunctionType.Exp, scale=SCALE
                    )
                    return p_t

                def v_quarter(q):
                    """project + transpose vx quarter q into vsbh[q//2]."""
                    h = q // 2
                    if q % 2 == 0:
                        nc.vector.memset(vsbh[h][:, :, DK : DK + 1], 1.0)
                    project_block(wv, 0, vxb[q], vTq[q])
                    t0 = (q % 2) * 4
                    ps = psM.tile([128, 4, 128], BF16, tag="ps_misc")
                    for tl in range(4):
                        nc.tensor.transpose(
                            ps[:, tl, :], vTq[q][:, tl * 128 : (tl + 1) * 128],
                            ident
                        )
                    nc.vector.tensor_copy(vsbh[h][:, t0 : t0 + 4, 0:DK], ps)

                o_big = [
                    o_pool.tile([128, 4, DK], F32, tag=f"ob{b}", name=f"ob{b}", bufs=1)
                    for b in range(2)
                ]
                out4 = out.rearrange("(b p q) k -> b p q k", q=4, p=128)

                def div_out(blk, qs, ps_o):
                    rec = o_pool.tile([128, 1], F32, tag="rec")
                    nc.vector.reciprocal(rec, ps_o[:, DK : DK + 1])
                    nc.vector.tensor_scalar_mul(o_big[blk][:, qs, :], ps_o[:, 0:DK], rec)
                    if qs == 3 and blk == 0:
                        nc.sync.dma_start(out=out4[0], in_=o_big[0])

                def pv(ps_o, p_pairs, qs, jset, h, start, stop):
                    j0 = jset[0] if isinstance(jset, list) else jset.start
                    for n, j in enumerate(jset):
                        nc.tensor.matmul(
                            ps_o,
                            p_pairs[(j - j0) // 2][
                                :, (j % 2) * 512 + qs * 128 : (j % 2) * 512 + (qs + 1) * 128
                            ],
                            vsbh[h][:, j % NCH, :],
                            start=(start and n == 0),
                            stop=(stop and n == len(jset) - 1),
                        )

                # ---------- pipeline ----------
                # tile_wait_until hints tell the static scheduler when each
                # stage's DMA data really lands (measured ETAs), so the
                # in-order engine streams don't park early-emitted work in
                # front of later work whose inputs arrive sooner.
                W = tc.tile_wait_until
                with nc.named_scope("q_proj"):
                    with W(0.007):
                        project_dr(wqk, 0, qxb[0], qTb[0])
                with W(0.0065):
                    project_dr(wqk, DK, kxp[0], kT[0], w=256)
                p0, p1, p1b = [], [], []
                with nc.named_scope("sc_a"):
                    with W(0.0075):
                        p0.append(scores_pair(0, 0, True))
                        gen_masks([0, 2])
                    with W(0.008):
                        project_dr(wqk, 0, qxb[1], qTb[1])
                        project_dr(wqk, DK, kxp[1], kT[1], w=256)
                    with W(0.0085):
                        p1.append(scores_pair(1, 0, False))
                    with W(0.009):
                        project_dr(wqk, DK, kxp[2], kT[2], w=256)
                        p0.append(scores_pair(0, 2, True))
                        gen_masks([4, 6])
                    with W(0.010):
                        p1.append(scores_pair(1, 2, False))
                # scalar's sixth load issued here: its HWDGE ring has free
                # slots by now and the engine sits idle between exps anyway
                nc.scalar.dma_start(out=vxb[1][:, 0:4, :], in_=vx_in[:, 1, 0:4])
                with nc.named_scope("sc_b"):
                    with W(0.0105):
                        project_dr(wqk, DK, kxp[3], kT[3], w=256)
                        p0.append(scores_pair(0, 4, True))
                    with W(0.0115):
                        p1.append(scores_pair(1, 4, False))
                    with W(0.012):
                        project_dr(wqk, DK, kxp[4], kT[4], w=256)
                        p0.append(scores_pair(0, 6, True))
                    with W(0.013):
                        p1.append(scores_pair(1, 6, False))
                    with W(0.018):
                        for n in range(4):
                            nc.vector.tensor_mul(p0[n], p0[n], masks_sb[2 * n])
                ps_o0 = [psO.tile([128, VW], F32, tag="out", name=f"ps_o0_{i}") for i in range(4)]
                ps_o1 = [psO.tile([128, VW], F32, tag="out", name=f"ps_o1_{i}") for i in range(4)]
                with nc.named_scope("sc_c"):
                    with W(0.013):
                        project_dr(wqk, DK, kxp[5], kT[5], w=256)
                    with W(0.014):
                        p1b.append(scores_pair(1, 8, True))
                        gen_masks([8, 10, 12, 14])
                    with W(0.0145):
                        project_dr(wqk, DK, kxp[6], kT[6], w=256)
                    with W(0.015):
                        p1b.append(scores_pair(1, 10, True))
                    with W(0.016):
                        project_dr(wqk, DK, kxp[7], kT[7], w=256)
                    with W(0.017):
                        p1b.append(scores_pair(1, 12, True))
                    with W(0.018):
                        p1b.append(scores_pair(1, 14, True))
                    with W(0.022):
                        for n in range(4):
                            nc.vector.tensor_mul(p1b[n], p1b[n], masks_sb[8 + 2 * n])
                with nc.named_scope("vnat"):
                    with W(0.011):
                        v_quarter(0)
                    with W(0.012):
                        v_quarter(1)
                with nc.named_scope("pv01"):
                    with W(0.013):
                        for qs in range(4):
                            pv(ps_o0[qs], p0, qs, range(NJ0), 0, True, True)
                            div_out(0, qs, ps_o0[qs])
                with nc.named_scope("vnat2"):
                    with W(0.0135):
                        v_quarter(2)
                with nc.named_scope("pv1"):
                    with W(0.0145):
                        for qs in range(4):
                            pv(ps_o1[qs], p1, qs, range(NCH), 0, True, False)
                with nc.named_scope("vnat3"):
                    with W(0.015):
                        v_quarter(3)
                with nc.named_scope("pv1b"):
                    with W(0.018):
                        for qs in range(4):
                            pv(ps_o1[qs], p1b, qs, range(NCH, NJ1), 1, False, True)
                            div_out(1, qs, ps_o1[qs])
                with nc.named_scope("store1"):
                    nc.scalar.dma_start(out=out4[1, :, 0:2], in_=o_big[1][:, 0:2])
                    nc.sync.dma_start(out=out4[1, :, 2:4], in_=o_big[1][:, 2:4])

    nc.compile()
    _CACHE["nc"] = nc
    return nc


def _shift_block(h):
    """[128, 16] f32: mask[p, t, c] = (c >= shift) == (key 128t+p <= query qb+c)."""
    qbase = (0, 1536) if h == 0 else (512, 1024)
    p = np.arange(128, dtype=np.float32)[:, None]
    t = np.arange(16, dtype=np.float32)[None, :]
    qb = np.where(t < NJ0, qbase[0], qbase[1])
    return (128.0 * t + p - qb).astype(np.float32)


def _blocked(arr, nblk, dtype):
    """[DM, ncols] -> [128, nblk, NCH, ncols//nblk] matching the SBUF tiles."""
    w = arr.shape[1] // nblk
    return np.ascontiguousarray(
        arr.reshape(NCH, 128, nblk, w).transpose(1, 2, 0, 3)
    ).astype(dtype)


def kernel(**inputs):
    queries = np.asarray(inputs["queries"], dtype=np.float32)
    keys = np.asarray(inputs["keys"], dtype=np.float32)
    values = np.asarray(inputs["values"], dtype=np.float32)

    nc = _build()
    f8 = ml_dtypes.float8_e4m3fn
    bf = ml_dtypes.bfloat16
    shifts = [_shift_block(0), _shift_block(1)]
    qrows = [np.r_[0:512, 1536:2048], np.r_[512:1536]]
    wT = {
        nm: np.asarray(inputs[nm], dtype=np.float32).T
        for nm in ("Wq", "Wk", "Wv")
    }
    wqk = np.ascontiguousarray(
        np.concatenate([wT["Wq"], wT["Wk"]], axis=1).reshape(NCH, 128, 2 * DK)
        .transpose(1, 0, 2) * WSC
    ).astype(f8)
    wv = np.ascontiguousarray(
        wT["Wv"].reshape(NCH, 128, DK).transpose(1, 0, 2)
    ).astype(bf)
    kxs = [_blocked(keys[b].T, 8, f8) for b in range(B)]
    vxs = [_blocked(values[b].T, 4, bf) for b in range(B)]

    in_maps = []
    for c in range(NCORES):
        b, h = c // 2, c % 2
        in_maps.append(
            {
                "qx": _blocked(queries[b][qrows[h]].T, 2, f8),
                "kx": kxs[b],
                "vx": vxs[b],
                "wqk": wqk,
                "wv": wv,
                "shifts": shifts[h],
            }
        )

    res = bass_utils.run_bass_kernel_spmd(
        nc, in_maps, list(range(NCORES)), **_CACHE.get("run_kwargs", {})
    )
    _CACHE["last_result"] = res

    # store layout is (p q): dram row blk*512 + p*4 + qs <- query qs*128 + p
    r = np.arange(512)
    local_q = (r % 4) * 128 + r // 4  # query index within block at dram row r
    perm = np.concatenate([local_q, 512 + local_q])
    out = np.empty((B, S, DK), dtype=np.float32)
    for c in range(NCORES):
        b, h = c // 2, c % 2
        out[b][qrows[h][perm]] = res.results[c]["out"]
    return out
